# revision 22
# baseline (speedup 1.0000x reference)
"""NativeFP4Linear TRN2 kernel: out = x @ (dequant(weight_fp4)).T + bias.

dequant(W)[o, i] = W[o, i] / block_scales[o*256 + i//16] / tensor_scale

Strategy (8 NeuronCores, tensor-parallel over out_features, 512 rows/core):
  - Host: transpose each core's weight slice to [in=4096, out=512] (matmul
    contracts along the partition dim, so the weight must be partition=i).
  - Device per core:
      rec   = 1/block_scales  (DVE reciprocal_approx_fast, [128, 1024] layout)
      rec   -> hi + lo fp32r pieces (exact sum)
      ES    = one-hot fp32r matmuls broadcast rec rows into [128 i, 512 o]
              expanded-scale tiles (hi+lo accumulated -> bit-exact fp32 scales)
      wdeq  = wT * ES           (DVE tensor-tensor, fp32 -> fp32r)
      out  += xT_chunk.T @ wdeq (fp32r matmuls, K accumulated in PSUM fp32)
      out   = out * (1/tensor_scale) + bias
  - Host: concatenate the 8 [32, 512] results -> [32, 4096].
"""
import numpy as np
from contextlib import ExitStack

import concourse.bass as bass
import concourse.mybir as mybir
import concourse.tile as tile
from concourse import bacc
from concourse.bass_utils import run_bass_kernel_spmd

F32 = mybir.dt.float32
F32R = mybir.dt.float32r
BF16 = mybir.dt.bfloat16
U8 = mybir.dt.uint8

N_CORES = 8
B = 32             # batch
I = 4096           # in_features
O = 4096           # out_features
OC = O // N_CORES  # out features per core = 512
BS = 16            # fp4 block size
NBLK = I // BS     # block-columns per output row = 256
NSUB = I // 128    # 128-row contraction sub-chunks = 32
SUB_PER_IT = 3     # sub-chunks fused per pipeline iteration
PREFETCH = 8

_CACHE = {}


def _build(inv_ts: float):
    nc = bacc.Bacc("TRN2", target_bir_lowering=False, debug=False,
                   enable_asserts=True, num_devices=N_CORES)

    wt = nc.dram_tensor("wt", [I, OC], F32, kind="ExternalInput").ap()
    sc = nc.dram_tensor("sc", [128, 1024], F32, kind="ExternalInput").ap()
    e16in = nc.dram_tensor("e16", [128, 16 * 128], BF16,
                           kind="ExternalInput").ap()
    # combo holds xt (fp32, 4096 B/partition)
    combo = nc.dram_tensor("combo", [128, 4096], U8, kind="ExternalInput").ap()
    biasb = nc.dram_tensor("biasb", [B, OC], F32, kind="ExternalInput").ap()
    out = nc.dram_tensor("out", [B, OC], F32, kind="ExternalOutput").ap()

    with tile.TileContext(nc) as tc, ExitStack() as ctx:
        cpool = ctx.enter_context(tc.tile_pool(name="const", bufs=1))
        wpool = ctx.enter_context(tc.tile_pool(name="w", bufs=10))
        dqpool = ctx.enter_context(tc.tile_pool(name="dq", bufs=3))
        espool = ctx.enter_context(tc.tile_pool(name="es", bufs=2, space="PSUM"))
        mpool = ctx.enter_context(tc.tile_pool(name="acc", bufs=1, space="PSUM"))

        # First iteration small so the first dequant starts early; last
        # iteration tiny so little work trails the final weight DMA.
        sizes = [1, 2] + [SUB_PER_IT] * 9 + [1, 1]
        assert sum(sizes) == NSUB
        starts = [sum(sizes[:i]) for i in range(len(sizes))]
        n_it = len(starts)

        # ---- setup DMAs (sc heads the reciprocal critical path) ----
        t_sc = cpool.tile([128, 1024], F32)
        nc.sync.dma_start(t_sc[:], sc[:])
        t_e16bf = cpool.tile([128, 16 * 128], BF16)
        nc.sync.dma_start(t_e16bf[:], e16in[:])
        t_combo = cpool.tile([128, 4096], U8)
        combo_inst = nc.sync.dma_start(t_combo[:], combo[:])

        def dma_w(t):
            # weight DMAs ride the Scalar HWDGE ring so their issue cost
            # doesn't serialize against the setup DMAs on the Sync ring.
            # All but the first are held until the setup data has landed:
            # the SDMA engines round-robin queues at packet granularity, so
            # concurrent bulk weight traffic would starve the small setup
            # transfers that gate the whole compute pipeline.
            g, nsc = starts[t], sizes[t]
            t_w = wpool.tile([128, SUB_PER_IT * OC], F32, tag="w")
            src = wt[g * 128:g * 128 + nsc * 128, :].rearrange(
                "(q p) n -> p q n", p=128)
            inst = nc.scalar.dma_start(t_w[:, :nsc * OC].rearrange(
                "p (q n) -> p q n", q=nsc), src)
            if t > 0:
                tile.add_dep_helper(inst.ins, combo_inst.ins,
                                    reason="hold bulk weights behind setup")
            return t_w

        w_tiles = [dma_w(t) for t in range(min(PREFETCH, n_it))]

        t_biasb = cpool.tile([B, OC], F32)
        nc.sync.dma_start(t_biasb[:], biasb[:])

        # ---- reciprocal chain, split in halves so the first expansion
        # matmuls only wait for the first half ----
        rhi, rlo = [], []
        for h in range(2):
            t_rec = cpool.tile([128, 512], F32, tag=f"rec{h}")
            nc.vector.reciprocal_approx_fast(t_rec[:], t_sc[:, 512 * h:512 * (h + 1)])
            t_rhi = cpool.tile([128, 512], F32R, tag=f"rhi{h}")
            nc.vector.tensor_copy(t_rhi[:], t_rec[:])
            t_rlo = cpool.tile([128, 512], F32R, tag=f"rlo{h}")
            nc.vector.tensor_sub(t_rlo[:], t_rec[:], t_rhi[:].bitcast(F32))
            rhi.append(t_rhi)
            rlo.append(t_rlo)
            if h == 0:
                t_e16 = cpool.tile([128, 16 * 128], F32R)
                nc.vector.tensor_copy(t_e16[:], t_e16bf[:])

        t_xtr = cpool.tile([128, NSUB * B], F32R)
        nc.vector.tensor_copy(t_xtr[:], t_combo[:, :4096].bitcast(F32))

        t_acc = mpool.tile([B, OC], F32)

        def emit_es(t):
            g, nsc = starts[t], sizes[t]
            t_es = espool.tile([128, SUB_PER_IT * OC], F32, tag="es")
            for j in range(nsc):
                gg = g + j
                v, u = gg % 16, gg // 16
                lhs = t_e16[:, 128 * v:128 * (v + 1)]
                dst = t_es[:, OC * j:OC * (j + 1)]
                nc.tensor.matmul(dst, lhs, rhi[u][:], start=True, stop=False)
                nc.tensor.matmul(dst, lhs, rlo[u][:], start=False, stop=True)
            return t_es

        # ---- software-pipelined main loop ----
        # PE order: ES(t+1) is emitted before main(t) so the tensor engine
        # fills the DVE-dequant latency with the next chunk's expansion.
        es_tiles = {0: emit_es(0)}
        for t in range(n_it):
            g, nsc = starts[t], sizes[t]
            if t + PREFETCH < n_it:
                w_tiles.append(dma_w(t + PREFETCH))
            if t + 1 < n_it:
                es_tiles[t + 1] = emit_es(t + 1)

            t_es = es_tiles.pop(t)
            t_w = w_tiles[t]
            t_dq = dqpool.tile([128, SUB_PER_IT * OC], F32R, tag="dq")
            nc.vector.tensor_mul(t_dq[:, :nsc * OC], t_w[:, :nsc * OC],
                                 t_es[:, :nsc * OC])

            for j in range(nsc):
                gg = g + j
                nc.tensor.matmul(t_acc[:], t_xtr[:, B * gg:B * (gg + 1)],
                                 t_dq[:, OC * j:OC * (j + 1)],
                                 start=(gg == 0), stop=(gg == NSUB - 1))

        # ---- epilogue: out = acc * (1/ts) + bias ----
        t_out = cpool.tile([B, OC], F32)
        nc.vector.scalar_tensor_tensor(
            t_out[:], t_acc[:], float(inv_ts), t_biasb[:],
            op0=mybir.AluOpType.mult, op1=mybir.AluOpType.add)
        nc.scalar.dma_start(out[:], t_out[:])

    nc.compile()
    return nc


def _host_prep(x, weight_fp4, block_scales, bias):
    """Build the per-core input maps."""
    import ml_dtypes
    x = np.asarray(x, dtype=np.float32)
    weight_fp4 = np.asarray(weight_fp4, dtype=np.float32)
    block_scales = np.asarray(block_scales, dtype=np.float32)
    bias = np.asarray(bias, dtype=np.float32)

    # x.T tiled: xt[p, 32 g + b] = x[b, 128 g + p]
    xt = np.ascontiguousarray(
        x.T.reshape(NSUB, 128, B).transpose(1, 0, 2).reshape(128, NSUB * B))

    # one-hot selectors: e16[k, 128 v + p] = (k == 8 v + p // 16)
    e16 = np.zeros((128, 16 * 128), dtype=ml_dtypes.bfloat16)
    for v in range(16):
        p = np.arange(128)
        e16[8 * v + p // 16, 128 * v + p] = 1.0

    combo = np.ascontiguousarray(xt.view(np.uint8).reshape(128, 4096))

    bs2 = block_scales.reshape(O, NBLK)

    in_maps = []
    for c in range(N_CORES):
        o0 = c * OC
        wt_c = np.ascontiguousarray(weight_fp4[o0:o0 + OC, :].T)
        s_core = bs2[o0:o0 + OC, :].T  # [256 blk, 512 o]
        sc_c = np.ascontiguousarray(
            s_core.reshape(2, 128, OC).transpose(1, 0, 2).reshape(128, 1024))
        biasb_c = np.ascontiguousarray(
            np.broadcast_to(bias[o0:o0 + OC][None, :], (B, OC)))
        in_maps.append({
            "wt": wt_c, "sc": sc_c, "e16": e16, "combo": combo,
            "biasb": biasb_c,
        })
    return in_maps


def _get_program(inv_ts: float):
    key = ("nc", float(inv_ts))
    if key not in _CACHE:
        _CACHE[key] = _build(inv_ts)
    return _CACHE[key]


def kernel(x, weight_fp4, tensor_scale, block_scales, bias, **run_kwargs):
    inv_ts = 1.0 / float(np.asarray(tensor_scale).reshape(-1)[0])
    nc = _get_program(inv_ts)
    in_maps = _host_prep(x, weight_fp4, block_scales, bias)
    res = run_bass_kernel_spmd(nc, in_maps, core_ids=list(range(N_CORES)),
                               **run_kwargs)
    out = np.empty((B, O), dtype=np.float32)
    for c in range(N_CORES):
        out[:, c * OC:(c + 1) * OC] = res.results[c]["out"]
    if run_kwargs.get("trace"):
        kernel.last_exec_time_ns = res.exec_time_ns
    return out


# revision 25
# speedup vs baseline: 1.0802x; 1.0802x over previous
"""NativeFP4Linear TRN2 kernel: out = x @ (dequant(weight_fp4)).T + bias.

dequant(W)[o, i] = W[o, i] / block_scales[o*256 + i//16] / tensor_scale

Strategy (8 NeuronCores, tensor-parallel over out_features, 512 rows/core):
  - Host: transpose each core's weight slice to [in=4096, out=512] (matmul
    contracts along the partition dim, so the weight must be partition=i).
  - Device per core:
      rec   = 1/block_scales  (DVE reciprocal_approx_fast, [128, 1024] layout)
      rec   -> hi + lo fp32r pieces (exact sum)
      ES    = one-hot fp32r matmuls broadcast rec rows into [128 i, 512 o]
              expanded-scale tiles (hi+lo accumulated -> bit-exact fp32 scales)
      wdeq  = wT * ES           (DVE tensor-tensor, fp32 -> fp32r)
      out  += xT_chunk.T @ wdeq (fp32r matmuls, K accumulated in PSUM fp32)
      out   = out * (1/tensor_scale) + bias
  - Host: concatenate the 8 [32, 512] results -> [32, 4096].
"""
import numpy as np
from contextlib import ExitStack

import concourse.bass as bass
import concourse.mybir as mybir
import concourse.tile as tile
from concourse import bacc
from concourse.bass_utils import run_bass_kernel_spmd

F32 = mybir.dt.float32
F32R = mybir.dt.float32r
BF16 = mybir.dt.bfloat16
U8 = mybir.dt.uint8

N_CORES = 8
B = 32             # batch
I = 4096           # in_features
O = 4096           # out_features
OC = O // N_CORES  # out features per core = 512
BS = 16            # fp4 block size
NBLK = I // BS     # block-columns per output row = 256
NSUB = I // 128    # 128-row contraction sub-chunks = 32
SUB_PER_IT = 3     # sub-chunks fused per pipeline iteration
PREFETCH = 8

_CACHE = {}


def _build(inv_ts: float):
    nc = bacc.Bacc("TRN2", target_bir_lowering=False, debug=False,
                   enable_asserts=True, num_devices=N_CORES)

    wt = nc.dram_tensor("wt", [I, OC], F32, kind="ExternalInput").ap()
    sc = nc.dram_tensor("sc", [128, 1024], F32, kind="ExternalInput").ap()
    e16in = nc.dram_tensor("e16", [128, 16 * 128], BF16,
                           kind="ExternalInput").ap()
    # combo holds xt (fp32, 4096 B/partition)
    combo = nc.dram_tensor("combo", [128, 4096], U8, kind="ExternalInput").ap()
    biasb = nc.dram_tensor("biasb", [B, OC], F32, kind="ExternalInput").ap()
    out = nc.dram_tensor("out", [B, OC], F32, kind="ExternalOutput").ap()

    with tile.TileContext(nc) as tc, ExitStack() as ctx:
        cpool = ctx.enter_context(tc.tile_pool(name="const", bufs=1))
        wpool = ctx.enter_context(tc.tile_pool(name="w", bufs=10))
        dqpool = ctx.enter_context(tc.tile_pool(name="dq", bufs=3))
        espool = ctx.enter_context(tc.tile_pool(name="es", bufs=2, space="PSUM"))
        mpool = ctx.enter_context(tc.tile_pool(name="acc", bufs=1, space="PSUM"))

        # First iteration small so the first dequant starts early; last
        # iteration tiny so little work trails the final weight DMA.
        sizes = [1, 2] + [SUB_PER_IT] * 9 + [1, 1]
        assert sum(sizes) == NSUB
        starts = [sum(sizes[:i]) for i in range(len(sizes))]
        n_it = len(starts)

        # ---- setup DMAs (e16 + sc head the critical path) ----
        t_e16bf = cpool.tile([128, 16 * 128], BF16)
        nc.sync.dma_start(t_e16bf[:], e16in[:])
        t_sc = cpool.tile([128, 1024], F32)
        nc.sync.dma_start(t_sc[:], sc[:])
        t_combo = cpool.tile([128, 4096], U8)
        combo_inst = nc.sync.dma_start(t_combo[:], combo[:])

        def dma_w(t):
            # weight DMAs ride the Scalar HWDGE ring so their issue cost
            # doesn't serialize against the setup DMAs on the Sync ring.
            # All but the first are held until the setup data has landed:
            # the SDMA engines round-robin queues at packet granularity, so
            # concurrent bulk weight traffic would starve the small setup
            # transfers that gate the whole compute pipeline.
            g, nsc = starts[t], sizes[t]
            t_w = wpool.tile([128, SUB_PER_IT * OC], F32, tag="w")
            src = wt[g * 128:g * 128 + nsc * 128, :].rearrange(
                "(q p) n -> p q n", p=128)
            inst = nc.scalar.dma_start(t_w[:, :nsc * OC].rearrange(
                "p (q n) -> p q n", q=nsc), src)
            if t > 0:
                tile.add_dep_helper(inst.ins, combo_inst.ins,
                                    reason="hold bulk weights behind setup")
            return t_w

        w_tiles = [dma_w(t) for t in range(min(PREFETCH, n_it))]

        t_biasb = cpool.tile([B, OC], F32)
        nc.sync.dma_start(t_biasb[:], biasb[:])

        # ---- PE warm-up: dummy matmuls on garbage data keep the tensor
        # engine's activity monitor busy so the first real expansion
        # matmuls run at 2.4 GHz instead of the cold 1.2 GHz ----
        warm_pool = ctx.enter_context(
            tc.tile_pool(name="warm", bufs=1, space="PSUM"))
        t_junk = cpool.tile([128, 64], BF16)
        nc.gpsimd.memset(t_junk[:], 1.0)
        t_wps = warm_pool.tile([128, 64], F32)
        for _ in range(56):
            nc.tensor.matmul(t_wps[:32, :], t_junk[:, :32], t_junk[:, :],
                             start=True, stop=True)

        # ---- e16 cast first: it rides ahead of the scales DMA ----
        t_e16 = cpool.tile([128, 16 * 128], F32R)
        e16_cast = nc.vector.tensor_copy(t_e16[:], t_e16bf[:])

        # ---- reciprocal chain, split in halves so the first expansion
        # matmuls only wait for the first half ----
        rhi, rlo = [], []
        prev = e16_cast
        for h in range(2):
            t_rec = cpool.tile([128, 512], F32, tag=f"rec{h}")
            i0 = nc.vector.reciprocal_approx_fast(
                t_rec[:], t_sc[:, 512 * h:512 * (h + 1)])
            # ordering-only dep: keep the h=1 chain from being scheduled
            # ahead of the h=0 chain on the DVE
            tile.add_dep_helper(i0.ins, prev.ins, sync=False,
                                reason="dve setup order")
            t_rhi = cpool.tile([128, 512], F32R, tag=f"rhi{h}")
            nc.vector.tensor_copy(t_rhi[:], t_rec[:])
            t_rlo = cpool.tile([128, 512], F32R, tag=f"rlo{h}")
            prev = nc.vector.tensor_sub(t_rlo[:], t_rec[:],
                                        t_rhi[:].bitcast(F32))
            rhi.append(t_rhi)
            rlo.append(t_rlo)

        t_xtr = cpool.tile([128, NSUB * B], F32R)
        nc.vector.tensor_copy(t_xtr[:], t_combo[:].bitcast(F32))

        t_acc = mpool.tile([B, OC], F32)

        def emit_es(t):
            g, nsc = starts[t], sizes[t]
            t_es = espool.tile([128, SUB_PER_IT * OC], F32, tag="es")
            for j in range(nsc):
                gg = g + j
                v, u = gg % 16, gg // 16
                lhs = t_e16[:, 128 * v:128 * (v + 1)]
                dst = t_es[:, OC * j:OC * (j + 1)]
                nc.tensor.matmul(dst, lhs, rhi[u][:], start=True, stop=False)
                nc.tensor.matmul(dst, lhs, rlo[u][:], start=False, stop=True)
            return t_es

        # ---- software-pipelined main loop ----
        # PE order: ES(t+1) is emitted before main(t) so the tensor engine
        # fills the DVE-dequant latency with the next chunk's expansion.
        es_tiles = {0: emit_es(0)}
        for t in range(n_it):
            g, nsc = starts[t], sizes[t]
            if t + PREFETCH < n_it:
                w_tiles.append(dma_w(t + PREFETCH))
            if t + 1 < n_it:
                es_tiles[t + 1] = emit_es(t + 1)

            t_es = es_tiles.pop(t)
            t_w = w_tiles[t]
            t_dq = dqpool.tile([128, SUB_PER_IT * OC], F32R, tag="dq")
            nc.vector.tensor_mul(t_dq[:, :nsc * OC], t_w[:, :nsc * OC],
                                 t_es[:, :nsc * OC])

            for j in range(nsc):
                gg = g + j
                nc.tensor.matmul(t_acc[:], t_xtr[:, B * gg:B * (gg + 1)],
                                 t_dq[:, OC * j:OC * (j + 1)],
                                 start=(gg == 0), stop=(gg == NSUB - 1))

        # ---- epilogue: out = acc * (1/ts) + bias ----
        t_out = cpool.tile([B, OC], F32)
        nc.vector.scalar_tensor_tensor(
            t_out[:], t_acc[:], float(inv_ts), t_biasb[:],
            op0=mybir.AluOpType.mult, op1=mybir.AluOpType.add)
        nc.scalar.dma_start(out[:], t_out[:])

    nc.compile()
    return nc


def _host_prep(x, weight_fp4, block_scales, bias):
    """Build the per-core input maps."""
    import ml_dtypes
    x = np.asarray(x, dtype=np.float32)
    weight_fp4 = np.asarray(weight_fp4, dtype=np.float32)
    block_scales = np.asarray(block_scales, dtype=np.float32)
    bias = np.asarray(bias, dtype=np.float32)

    # x.T tiled: xt[p, 32 g + b] = x[b, 128 g + p]
    xt = np.ascontiguousarray(
        x.T.reshape(NSUB, 128, B).transpose(1, 0, 2).reshape(128, NSUB * B))

    # one-hot selectors: e16[k, 128 v + p] = (k == 8 v + p // 16)
    e16 = np.zeros((128, 16 * 128), dtype=ml_dtypes.bfloat16)
    for v in range(16):
        p = np.arange(128)
        e16[8 * v + p // 16, 128 * v + p] = 1.0

    combo = np.ascontiguousarray(xt.view(np.uint8).reshape(128, 4096))

    bs2 = block_scales.reshape(O, NBLK)

    in_maps = []
    for c in range(N_CORES):
        o0 = c * OC
        wt_c = np.ascontiguousarray(weight_fp4[o0:o0 + OC, :].T)
        s_core = bs2[o0:o0 + OC, :].T  # [256 blk, 512 o]
        sc_c = np.ascontiguousarray(
            s_core.reshape(2, 128, OC).transpose(1, 0, 2).reshape(128, 1024))
        biasb_c = np.ascontiguousarray(
            np.broadcast_to(bias[o0:o0 + OC][None, :], (B, OC)))
        in_maps.append({
            "wt": wt_c, "sc": sc_c, "e16": e16, "combo": combo,
            "biasb": biasb_c,
        })
    return in_maps


def _get_program(inv_ts: float):
    key = ("nc", float(inv_ts))
    if key not in _CACHE:
        _CACHE[key] = _build(inv_ts)
    return _CACHE[key]


def kernel(x, weight_fp4, tensor_scale, block_scales, bias, **run_kwargs):
    inv_ts = 1.0 / float(np.asarray(tensor_scale).reshape(-1)[0])
    nc = _get_program(inv_ts)
    in_maps = _host_prep(x, weight_fp4, block_scales, bias)
    res = run_bass_kernel_spmd(nc, in_maps, core_ids=list(range(N_CORES)),
                               **run_kwargs)
    out = np.empty((B, O), dtype=np.float32)
    for c in range(N_CORES):
        out[:, c * OC:(c + 1) * OC] = res.results[c]["out"]
    if run_kwargs.get("trace"):
        kernel.last_exec_time_ns = res.exec_time_ns
    return out


# revision 27
# speedup vs baseline: 1.1223x; 1.0389x over previous
"""NativeFP4Linear TRN2 kernel: out = x @ (dequant(weight_fp4)).T + bias.

dequant(W)[o, i] = W[o, i] / block_scales[o*256 + i//16] / tensor_scale

Strategy (8 NeuronCores, tensor-parallel over out_features, 512 rows/core):
  - Host: transpose each core's weight slice to [in=4096, out=512] (matmul
    contracts along the partition dim, so the weight must be partition=i).
  - Device per core:
      rec   = 1/block_scales  (DVE reciprocal_approx_fast, [128, 1024] layout)
      rec   -> hi + lo fp32r pieces (exact sum)
      ES    = one-hot fp32r matmuls broadcast rec rows into [128 i, 512 o]
              expanded-scale tiles (hi+lo accumulated -> bit-exact fp32 scales)
      wdeq  = wT * ES           (DVE tensor-tensor, fp32 -> fp32r)
      out  += xT_chunk.T @ wdeq (fp32r matmuls, K accumulated in PSUM fp32)
      out   = out * (1/tensor_scale) + bias
  - Host: concatenate the 8 [32, 512] results -> [32, 4096].
"""
import numpy as np
from contextlib import ExitStack

import concourse.bass as bass
import concourse.mybir as mybir
import concourse.tile as tile
from concourse import bacc
from concourse.bass_utils import run_bass_kernel_spmd

F32 = mybir.dt.float32
F32R = mybir.dt.float32r
BF16 = mybir.dt.bfloat16
U8 = mybir.dt.uint8

N_CORES = 8
B = 32             # batch
I = 4096           # in_features
O = 4096           # out_features
OC = O // N_CORES  # out features per core = 512
BS = 16            # fp4 block size
NBLK = I // BS     # block-columns per output row = 256
NSUB = I // 128    # 128-row contraction sub-chunks = 32
SUB_PER_IT = 3     # sub-chunks fused per pipeline iteration
PREFETCH = 8

_CACHE = {}


def _build(inv_ts: float):
    nc = bacc.Bacc("TRN2", target_bir_lowering=False, debug=False,
                   enable_asserts=True, num_devices=N_CORES)

    wt = nc.dram_tensor("wt", [I, OC], F32, kind="ExternalInput").ap()
    sc = nc.dram_tensor("sc", [128, 1024], F32, kind="ExternalInput").ap()
    e16in = nc.dram_tensor("e16", [128, 16 * 128], BF16,
                           kind="ExternalInput").ap()
    # combo holds xt (fp32, 4096 B/partition)
    combo = nc.dram_tensor("combo", [128, 4096], U8, kind="ExternalInput").ap()
    biasb = nc.dram_tensor("biasb", [B, OC], F32, kind="ExternalInput").ap()
    out = nc.dram_tensor("out", [B, OC], F32, kind="ExternalOutput").ap()

    with tile.TileContext(nc) as tc, ExitStack() as ctx:
        cpool = ctx.enter_context(tc.tile_pool(name="const", bufs=1))
        wpool = ctx.enter_context(tc.tile_pool(name="w", bufs=10))
        dqpool = ctx.enter_context(tc.tile_pool(name="dq", bufs=3))
        espool = ctx.enter_context(tc.tile_pool(name="es", bufs=2, space="PSUM"))
        mpool = ctx.enter_context(tc.tile_pool(name="acc", bufs=1, space="PSUM"))

        # First iteration small so the first dequant starts early; last
        # iteration tiny so little work trails the final weight DMA.
        sizes = [1, 2] + [SUB_PER_IT] * 9 + [1, 1]
        assert sum(sizes) == NSUB
        starts = [sum(sizes[:i]) for i in range(len(sizes))]
        n_it = len(starts)

        # ---- setup DMAs (e16 + sc head the critical path; separate HWDGE
        # rings so their streams run in parallel) ----
        t_sc = cpool.tile([128, 1024], F32)
        nc.sync.dma_start(t_sc[:], sc[:])
        t_e16bf = cpool.tile([128, 16 * 128], BF16)
        nc.scalar.dma_start(t_e16bf[:], e16in[:])
        t_combo = cpool.tile([128, 4096], U8)
        combo_inst = nc.sync.dma_start(t_combo[:], combo[:])

        def dma_w(t):
            # weight DMAs ride the Scalar HWDGE ring so their issue cost
            # doesn't serialize against the setup DMAs on the Sync ring.
            # All but the first are held until the setup data has landed:
            # the SDMA engines round-robin queues at packet granularity, so
            # concurrent bulk weight traffic would starve the small setup
            # transfers that gate the whole compute pipeline.
            g, nsc = starts[t], sizes[t]
            t_w = wpool.tile([128, SUB_PER_IT * OC], F32, tag="w")
            src = wt[g * 128:g * 128 + nsc * 128, :].rearrange(
                "(q p) n -> p q n", p=128)
            inst = nc.scalar.dma_start(t_w[:, :nsc * OC].rearrange(
                "p (q n) -> p q n", q=nsc), src)
            if t > 0:
                tile.add_dep_helper(inst.ins, combo_inst.ins,
                                    reason="hold bulk weights behind setup")
            return t_w

        w_tiles = [dma_w(t) for t in range(min(PREFETCH, n_it))]

        t_biasb = cpool.tile([B, OC], F32)
        nc.sync.dma_start(t_biasb[:], biasb[:])

        # ---- PE warm-up: dummy matmuls on garbage data keep the tensor
        # engine's activity monitor busy so the first real expansion
        # matmuls run at 2.4 GHz instead of the cold 1.2 GHz ----
        warm_pool = ctx.enter_context(
            tc.tile_pool(name="warm", bufs=1, space="PSUM"))
        t_junk = cpool.tile([128, 64], BF16)
        nc.gpsimd.memset(t_junk[:], 1.0)
        t_wps = warm_pool.tile([128, 64], F32)
        for _ in range(90):
            nc.tensor.matmul(t_wps[:32, :], t_junk[:, :32], t_junk[:, :],
                             start=True, stop=True)

        # ---- e16 cast first: it rides ahead of the scales DMA ----
        t_e16 = cpool.tile([128, 16 * 128], F32R)
        e16_cast = nc.vector.tensor_copy(t_e16[:], t_e16bf[:])

        # ---- reciprocal chain, split in halves so the first expansion
        # matmuls only wait for the first half ----
        rhi, rlo = [], []
        prev = e16_cast
        for h in range(2):
            t_rec = cpool.tile([128, 512], F32, tag=f"rec{h}")
            i0 = nc.vector.reciprocal_approx_fast(
                t_rec[:], t_sc[:, 512 * h:512 * (h + 1)])
            # ordering-only dep: keep the h=1 chain from being scheduled
            # ahead of the h=0 chain on the DVE
            tile.add_dep_helper(i0.ins, prev.ins, sync=False,
                                reason="dve setup order")
            t_rhi = cpool.tile([128, 512], F32R, tag=f"rhi{h}")
            nc.vector.tensor_copy(t_rhi[:], t_rec[:])
            t_rlo = cpool.tile([128, 512], F32R, tag=f"rlo{h}")
            prev = nc.vector.tensor_sub(t_rlo[:], t_rec[:],
                                        t_rhi[:].bitcast(F32))
            rhi.append(t_rhi)
            rlo.append(t_rlo)

        t_xtr = cpool.tile([128, NSUB * B], F32R)
        nc.vector.tensor_copy(t_xtr[:], t_combo[:].bitcast(F32))

        t_acc = mpool.tile([B, OC], F32)

        def emit_es(t):
            g, nsc = starts[t], sizes[t]
            t_es = espool.tile([128, SUB_PER_IT * OC], F32, tag="es")
            for j in range(nsc):
                gg = g + j
                v, u = gg % 16, gg // 16
                lhs = t_e16[:, 128 * v:128 * (v + 1)]
                dst = t_es[:, OC * j:OC * (j + 1)]
                nc.tensor.matmul(dst, lhs, rhi[u][:], start=True, stop=False)
                nc.tensor.matmul(dst, lhs, rlo[u][:], start=False, stop=True)
            return t_es

        # ---- software-pipelined main loop ----
        # PE order: ES(t+1) is emitted before main(t) so the tensor engine
        # fills the DVE-dequant latency with the next chunk's expansion.
        es_tiles = {0: emit_es(0)}
        for t in range(n_it):
            g, nsc = starts[t], sizes[t]
            if t + PREFETCH < n_it:
                w_tiles.append(dma_w(t + PREFETCH))
            if t + 1 < n_it:
                es_tiles[t + 1] = emit_es(t + 1)

            t_es = es_tiles.pop(t)
            t_w = w_tiles[t]
            t_dq = dqpool.tile([128, SUB_PER_IT * OC], F32R, tag="dq")
            nc.vector.tensor_mul(t_dq[:, :nsc * OC], t_w[:, :nsc * OC],
                                 t_es[:, :nsc * OC])

            for j in range(nsc):
                gg = g + j
                nc.tensor.matmul(t_acc[:], t_xtr[:, B * gg:B * (gg + 1)],
                                 t_dq[:, OC * j:OC * (j + 1)],
                                 start=(gg == 0), stop=(gg == NSUB - 1))

        # ---- epilogue: out = acc * (1/ts) + bias ----
        t_out = cpool.tile([B, OC], F32)
        nc.vector.scalar_tensor_tensor(
            t_out[:], t_acc[:], float(inv_ts), t_biasb[:],
            op0=mybir.AluOpType.mult, op1=mybir.AluOpType.add)
        nc.scalar.dma_start(out[:], t_out[:])

    nc.compile()
    return nc


def _host_prep(x, weight_fp4, block_scales, bias):
    """Build the per-core input maps."""
    import ml_dtypes
    x = np.asarray(x, dtype=np.float32)
    weight_fp4 = np.asarray(weight_fp4, dtype=np.float32)
    block_scales = np.asarray(block_scales, dtype=np.float32)
    bias = np.asarray(bias, dtype=np.float32)

    # x.T tiled: xt[p, 32 g + b] = x[b, 128 g + p]
    xt = np.ascontiguousarray(
        x.T.reshape(NSUB, 128, B).transpose(1, 0, 2).reshape(128, NSUB * B))

    # one-hot selectors: e16[k, 128 v + p] = (k == 8 v + p // 16)
    e16 = np.zeros((128, 16 * 128), dtype=ml_dtypes.bfloat16)
    for v in range(16):
        p = np.arange(128)
        e16[8 * v + p // 16, 128 * v + p] = 1.0

    combo = np.ascontiguousarray(xt.view(np.uint8).reshape(128, 4096))

    bs2 = block_scales.reshape(O, NBLK)

    in_maps = []
    for c in range(N_CORES):
        o0 = c * OC
        wt_c = np.ascontiguousarray(weight_fp4[o0:o0 + OC, :].T)
        s_core = bs2[o0:o0 + OC, :].T  # [256 blk, 512 o]
        sc_c = np.ascontiguousarray(
            s_core.reshape(2, 128, OC).transpose(1, 0, 2).reshape(128, 1024))
        biasb_c = np.ascontiguousarray(
            np.broadcast_to(bias[o0:o0 + OC][None, :], (B, OC)))
        in_maps.append({
            "wt": wt_c, "sc": sc_c, "e16": e16, "combo": combo,
            "biasb": biasb_c,
        })
    return in_maps


def _get_program(inv_ts: float):
    key = ("nc", float(inv_ts))
    if key not in _CACHE:
        _CACHE[key] = _build(inv_ts)
    return _CACHE[key]


def kernel(x, weight_fp4, tensor_scale, block_scales, bias, **run_kwargs):
    inv_ts = 1.0 / float(np.asarray(tensor_scale).reshape(-1)[0])
    nc = _get_program(inv_ts)
    in_maps = _host_prep(x, weight_fp4, block_scales, bias)
    res = run_bass_kernel_spmd(nc, in_maps, core_ids=list(range(N_CORES)),
                               **run_kwargs)
    out = np.empty((B, O), dtype=np.float32)
    for c in range(N_CORES):
        out[:, c * OC:(c + 1) * OC] = res.results[c]["out"]
    if run_kwargs.get("trace"):
        kernel.last_exec_time_ns = res.exec_time_ns
    return out
